# revision 20
# baseline (speedup 1.0000x reference)
"""Trainium2 Bass kernel for nn_LinearCondensed.

Computes out[b, o] = sum_k weight[o, k] * x[b, indx_seqs[o, k]] + bias[o]
with B=2048, IN_F=OUT_F=4096, FAN_IN=32.

Strategy: the gather has no fast on-chip primitive (GPSIMD ap_gather measured
~20x below its modeled rate; DMA descriptor gather materializes 32x the data
of x), so we densify the sparse weight matrix on the host --
W'[o, i] = sum_{k: indx_seqs[o,k]==i} weight[o, k] -- and run a dense bf16
matmul out = x @ W'^T + bias on the PE array (1 cycle/row, same speed as
fp32r but half the DMA traffic; measured rel_err 3.0e-3 vs the 2e-2 gate;
fp8 DoubleRow is only 2x and fails the gate at 3-5e-2).

Sharding is 2x4: batch halves x out_feature quarters. Per core that is
8 b-tiles of x (8MB bf16), a 1024-column W' shard (8MB bf16), and a
1024-wide output (4MB fp32) -- 20.5MB vs 24.25MB for 1D out_feature
sharding. The PE cost is identical (512 matmuls x 512 rows ~ 110.6us,
the invariant dense-GEMM floor), but the lighter, earlier-finishing
stream reduces cross-core HBM contention jitter, and phase 1 is PE-bound
with growing stream slack (W k-tile arrives in 0.66us, PE consumes it in
0.86us) instead of knife-edge stream-paced.

Schedule per core: dummy matmuls warm the PE p-state during the ~7us
engine-boot head; the single sync HWDGE queue (measured ~390GB/s, and a
second queue only splits the same per-core bandwidth) streams x0, x1,
then W in 8 1MB groups (first split 1+3 so the first real matmul depends
on x0+x1+one k-tile), then x2+ just in time for the k-inner phase.
b-tiles 0-1 run k-outer across both 512-column PSUM groups. Bias is
folded into the PSUM drain (pre-replicated across partitions on host).
The last b-tile's two column groups are staggered so the first group's
drain overlaps the second group's final matmuls. Host pre-tiles both
operands into the exact SBUF layouts so every DMA is contiguous.
"""

import os
import sys
import types

import ml_dtypes
import numpy as np

import concourse.bacc as bacc
import concourse.mybir as mybir
import concourse.tile as tile
from concourse.bass_utils import run_bass_kernel_spmd

B, IN_F, OUT_F, FAN_IN = 2048, 4096, 4096, 32
NCORES = 8
MB_, MO = 2, 4                 # core grid: batch halves x outf quarters
OSH = OUT_F // MO              # 1024 output features per core
BSH = B // MB_                 # 1024 batch rows per core
P = 128                        # partitions
BT = BSH // P                  # 8 batch tiles per core
KT = IN_F // P                 # 32 contraction tiles
N = 512                        # moving columns per matmul (PSUM bank limit)
NG = OSH // N                  # 2 column groups per core

f32 = mybir.dt.float32
bf16 = mybir.dt.bfloat16

_cache = {}


def _enable_ntff_hook():
    """Register the ctypes NTFF profile hook (the image's antenv lacks
    axon_hooks); lets trace=True produce a neuron-profile under axon."""
    try:
        from antenv.axon_hooks import get_axon_ntff_profile_hook  # noqa: F401
        return
    except ImportError:
        pass
    try:
        import antenv
        from trn_agent_boot.trn_boot import _ntff_profile_via_ctypes

        mod = types.ModuleType("antenv.axon_hooks")
        holder = [None]
        mod.set_axon_ntff_profile_hook = lambda h: holder.__setitem__(0, h)
        mod.get_axon_ntff_profile_hook = lambda: holder[0]
        antenv.axon_hooks = mod
        sys.modules["antenv.axon_hooks"] = mod
        mod.set_axon_ntff_profile_hook(
            _ntff_profile_via_ctypes("/opt/axon/libaxon_pjrt.so"))
        import concourse.bass_utils as bu
        bu.upload_artifacts = lambda tmpdir: str(tmpdir)
    except Exception:
        pass


def _build():
    nc = bacc.Bacc()
    # Layouts (host-pretiled, all contiguous):
    #   XT[t, p, a, c] = x[b0 + t*128 + c, a*128 + p]  -> per b-tile: [128, KT*128]
    #   WT[a, p, n]    = W'[o0 + n, a*128 + p]         -> [KT, 128, 1024]
    XT = nc.declare_dram_parameter("XT", [BT, P, KT * P], bf16, isOutput=False)
    WT = nc.declare_dram_parameter("WT", [KT, P, OSH], bf16, isOutput=False)
    BIAS = nc.declare_dram_parameter("BIAS", [P, OSH], f32, isOutput=False)
    OUT = nc.declare_dram_parameter("OUT", [BSH, OSH], f32, isOutput=True)

    XTv = XT.ap().rearrange("t p (a c) -> t p a c", a=KT)

    with tile.TileContext(nc) as tc:
        with (
            tc.tile_pool(name="wpool", bufs=1) as wpool,
            tc.tile_pool(name="xpool", bufs=4) as xpool,
            tc.tile_pool(name="cpool", bufs=1) as cpool,
            tc.tile_pool(name="opool", bufs=3) as opool,
            tc.tile_pool(name="psum", bufs=4, space="PSUM") as psum,
        ):
            xtiles = {}

            # PE p-state warmup: real matmuls can't start until x0+x1+w_k0
            # land (~15us); dummies keep the array busy from engine boot so
            # it runs at full clock when real work arrives.
            dl = cpool.tile([P, P], bf16)
            dr = cpool.tile([P, N], bf16)
            nc.vector.memset(dl[:], 0)
            nc.vector.memset(dr[:], 0)
            dacc = psum.tile([P, N], f32, name="dacc", tag="dacc", bufs=1)
            for _ in range(20):
                nc.tensor.matmul(dacc[:], dl[:], dr[:], start=True, stop=True)

            def load_x(t):
                xs = xpool.tile([P, KT, P], bf16, tag="xs")
                nc.sync.dma_start(xs[:], XTv[t])
                xtiles[t] = xs

            load_x(0)
            load_x(1)
            # W in 8 groups of 4 k-tiles (1MB per DMA), first group split 1+3
            # so the first matmul's dependency is a single W k-tile.
            WG = 4
            WTv = WT.ap().rearrange("(g j) p n -> g p j n", j=WG)
            wgroups = []
            brow = None
            for g in range(KT // WG):
                w = wpool.tile([P, WG, OSH], bf16, tag=f"w{g}")
                if g == 0:
                    nc.sync.dma_start(w[:, 0:1, :], WTv[0][:, 0:1, :])
                    nc.sync.dma_start(w[:, 1:, :], WTv[0][:, 1:, :])
                else:
                    nc.sync.dma_start(w[:], WTv[g])
                wgroups.append(w)
                if g == 1:
                    brow = cpool.tile([P, OSH], f32)
                    nc.sync.dma_start(brow[:], BIAS[:])
            wtiles = [wgroups[a // WG][:, a % WG, :] for a in range(KT)]

            # bias folded into the PSUM drain (bias row pre-replicated
            # across partitions on host)
            def finish_group(t, gi, acc):
                osb = opool.tile([P, N], f32, tag="osb")
                nc.vector.tensor_tensor(
                    osb[:], acc[:], brow[:, gi * N:(gi + 1) * N],
                    mybir.AluOpType.add)
                nc.scalar.dma_start(
                    OUT.ap()[t * P:(t + 1) * P, gi * N:(gi + 1) * N], osb[:])

            # Phase 1: b-tiles 0-1, k-outer across both column groups, so the
            # PE consumes each W group as it lands (and with 4 matmuls per
            # k-tile vs 0.66us arrival, the stream slack grows every k-tile).
            G = 2
            accs = [[psum.tile([P, N], f32, name=f"acc{t}g{gi}", tag="acc")
                     for gi in range(NG)] for t in range(G)]
            for a in range(KT):
                for t in range(G):
                    for gi in range(NG):
                        nc.tensor.matmul(
                            accs[t][gi][:], xtiles[t][:, a, :],
                            wtiles[a][:, gi * N:(gi + 1) * N],
                            start=(a == 0), stop=(a == KT - 1),
                        )
            for t in range(G):
                for gi in range(NG):
                    finish_group(t, gi, accs[t][gi])

            # Phase 2: remaining b-tiles, k-inner, x streamed just in time.
            # Column group 0's final matmul lands NG matmuls before group 1's,
            # so its drain overlaps the tail -- on every tile, including the
            # last one.
            for t in range(G, BT):
                load_x(t)
                xsb = xtiles[t]
                gaccs = [psum.tile([P, N], f32, name=f"acc{t}g{gi}", tag="acc")
                         for gi in range(NG)]
                for a in range(KT):
                    for gi in range(NG):
                        nc.tensor.matmul(
                            gaccs[gi][:],
                            xsb[:, a, :],       # lhsT: [K=128 (i), M=128 (b)]
                            wtiles[a][:, gi * N:(gi + 1) * N],
                            start=(a == 0),
                            stop=(a == KT - 1),
                        )
                for gi in range(NG):
                    finish_group(t, gi, gaccs[gi])

    nc.compile()
    return nc


def kernel(x, weight, bias, indx_seqs):
    x = np.asarray(x, dtype=np.float32)
    weight = np.asarray(weight, dtype=np.float32)
    bias = np.asarray(bias, dtype=np.float32)
    indx_seqs = np.asarray(indx_seqs)

    if "nc" not in _cache:
        _cache["nc"] = _build()
    nc = _cache["nc"]

    # Densify sparse weights: W'[o, i] += weight[o, k] at i = indx_seqs[o, k]
    wd = np.zeros((OUT_F, IN_F), dtype=np.float32)
    np.add.at(wd, (np.arange(OUT_F)[:, None], indx_seqs), weight)

    # Host pre-tiling into SBUF layouts, cast to bf16 (PE runs bf16 at the
    # same 1 cycle/row as fp32r; measured rel_err 3.0e-3 vs the 2e-2 gate).
    # XT[t, p, a, c] = x[t*128+c, a*128+p], 16 global b-tiles.
    xt = np.ascontiguousarray(
        x.reshape(B // P, P, KT, P).transpose(0, 3, 2, 1)
    ).reshape(B // P, P, KT * P).astype(ml_dtypes.bfloat16)
    wts, biases = [], []
    for q in range(MO):
        wshard = wd[q * OSH:(q + 1) * OSH]            # (1024, 4096)
        # WT[a, p, n] = W'[o0+n, a*128+p]
        wts.append(np.ascontiguousarray(
            wshard.reshape(OSH, KT, P).transpose(1, 2, 0)).astype(ml_dtypes.bfloat16))
        biases.append(np.ascontiguousarray(
            np.broadcast_to(bias[q * OSH:(q + 1) * OSH], (P, OSH))))
    in_maps = []
    for c in range(NCORES):
        h, q = divmod(c, MO)
        in_maps.append({
            "XT": xt[h * BT:(h + 1) * BT],
            "WT": wts[q],
            "BIAS": biases[q],
        })

    trace = bool(int(os.environ.get("BASSK_TRACE", "0"))) or bool(
        os.environ.get("BASS_TRACE"))
    if trace:
        _enable_ntff_hook()
    res = run_bass_kernel_spmd(
        nc, in_maps, list(range(NCORES)), trace=trace,
        trace_cores=list(range(NCORES)) if trace else None,
    )
    _cache["last_results"] = res

    # Stitch the 2x4 grid back into the full (B, OUT_F) output.
    out = np.empty((B, OUT_F), dtype=np.float32)
    for c in range(NCORES):
        h, q = divmod(c, MO)
        out[h * BSH:(h + 1) * BSH, q * OSH:(q + 1) * OSH] = res.results[c]["OUT"]
    return out


# revision 24
# speedup vs baseline: 1.0175x; 1.0175x over previous
"""Trainium2 Bass kernel for nn_LinearCondensed.

Computes out[b, o] = sum_k weight[o, k] * x[b, indx_seqs[o, k]] + bias[o]
with B=2048, IN_F=OUT_F=4096, FAN_IN=32.

Strategy: the gather has no fast on-chip primitive (GPSIMD ap_gather measured
~20x below its modeled rate; DMA descriptor gather materializes 32x the data
of x), so we densify the sparse weight matrix on the host --
W'[o, i] = sum_{k: indx_seqs[o,k]==i} weight[o, k] -- and run a dense bf16
matmul out = x @ W'^T + bias on the PE array (1 cycle/row, same speed as
fp32r but half the DMA traffic; measured rel_err 3.0e-3 vs the 2e-2 gate;
fp8 DoubleRow is only 2x and fails the gate at 3-5e-2).

Sharding is 2x4: batch halves x out_feature quarters. Per core that is
8 b-tiles of x (8MB bf16), a 1024-column W' shard (8MB bf16), and a
1024-wide output (4MB fp32) -- 20.5MB vs 24.25MB for 1D out_feature
sharding. The PE cost is identical (512 matmuls x 512 rows ~ 110.6us,
the invariant dense-GEMM floor), but the lighter, earlier-finishing
stream reduces cross-core HBM contention jitter, and phase 1 is PE-bound
with growing stream slack (W k-tile arrives in 0.66us, PE consumes it in
0.86us) instead of knife-edge stream-paced.

Schedule per core: dummy matmuls warm the PE p-state during the ~7us
engine-boot head; the single sync HWDGE queue (measured ~390GB/s, and a
second queue only splits the same per-core bandwidth) streams x0, x1,
then W in 8 1MB groups (first split 1+3 so the first real matmul depends
on x0+x1+one k-tile), then x2+ just in time for the k-inner phase.
b-tiles 0-1 run k-outer across both 512-column PSUM groups. Bias is
folded into the PSUM drain (pre-replicated across partitions on host).
The last b-tile's two column groups are staggered so the first group's
drain overlaps the second group's final matmuls. Host pre-tiles both
operands into the exact SBUF layouts so every DMA is contiguous.
"""

import os
import sys
import types

import ml_dtypes
import numpy as np

import concourse.bacc as bacc
import concourse.mybir as mybir
import concourse.tile as tile
from concourse.bass_utils import run_bass_kernel_spmd

B, IN_F, OUT_F, FAN_IN = 2048, 4096, 4096, 32
NCORES = 8
MB_, MO = 2, 4                 # core grid: batch halves x outf quarters
OSH = OUT_F // MO              # 1024 output features per core
BSH = B // MB_                 # 1024 batch rows per core
P = 128                        # partitions
BT = BSH // P                  # 8 batch tiles per core
KT = IN_F // P                 # 32 contraction tiles
N = 512                        # moving columns per matmul (PSUM bank limit)
NG = OSH // N                  # 2 column groups per core

f32 = mybir.dt.float32
bf16 = mybir.dt.bfloat16

_cache = {}


def _enable_ntff_hook():
    """Register the ctypes NTFF profile hook (the image's antenv lacks
    axon_hooks); lets trace=True produce a neuron-profile under axon."""
    try:
        from antenv.axon_hooks import get_axon_ntff_profile_hook  # noqa: F401
        return
    except ImportError:
        pass
    try:
        import antenv
        from trn_agent_boot.trn_boot import _ntff_profile_via_ctypes

        mod = types.ModuleType("antenv.axon_hooks")
        holder = [None]
        mod.set_axon_ntff_profile_hook = lambda h: holder.__setitem__(0, h)
        mod.get_axon_ntff_profile_hook = lambda: holder[0]
        antenv.axon_hooks = mod
        sys.modules["antenv.axon_hooks"] = mod
        mod.set_axon_ntff_profile_hook(
            _ntff_profile_via_ctypes("/opt/axon/libaxon_pjrt.so"))
        import concourse.bass_utils as bu
        bu.upload_artifacts = lambda tmpdir: str(tmpdir)
    except Exception:
        pass


def _build():
    nc = bacc.Bacc()
    # Layouts (host-pretiled, all contiguous):
    #   XT[t, p, a, c] = x[b0 + t*128 + c, a*128 + p]  -> per b-tile: [128, KT*128]
    #   WT[a, p, n]    = W'[o0 + n, a*128 + p]         -> [KT, 128, 1024]
    XT = nc.declare_dram_parameter("XT", [BT, P, KT * P], bf16, isOutput=False)
    WT = nc.declare_dram_parameter("WT", [KT, P, OSH], bf16, isOutput=False)
    BIAS = nc.declare_dram_parameter("BIAS", [P, OSH], f32, isOutput=False)
    OUT = nc.declare_dram_parameter("OUT", [BSH, OSH], f32, isOutput=True)

    XTv = XT.ap().rearrange("t p (a c) -> t p a c", a=KT)

    with tile.TileContext(nc) as tc:
        with (
            tc.tile_pool(name="wpool", bufs=1) as wpool,
            tc.tile_pool(name="xpool", bufs=4) as xpool,
            tc.tile_pool(name="cpool", bufs=1) as cpool,
            tc.tile_pool(name="opool", bufs=3) as opool,
            tc.tile_pool(name="psum", bufs=4, space="PSUM") as psum,
        ):
            xtiles = {}

            # PE p-state warmup: real matmuls can't start until x0+x1+w_k0
            # land (~15us); dummies keep the array busy from engine boot so
            # it runs at full clock when real work arrives.
            dl = cpool.tile([P, P], bf16)
            dr = cpool.tile([P, N], bf16)
            nc.vector.memset(dl[:], 0)
            nc.vector.memset(dr[:], 0)
            dacc = psum.tile([P, N], f32, name="dacc", tag="dacc", bufs=1)
            for _ in range(20):
                nc.tensor.matmul(dacc[:], dl[:], dr[:], start=True, stop=True)

            def load_x(t):
                xs = xpool.tile([P, KT, P], bf16, tag="xs")
                nc.sync.dma_start(xs[:], XTv[t])
                xtiles[t] = xs

            load_x(0)
            load_x(1)
            # W in 8 groups of 4 k-tiles (1MB per DMA), first group split 1+3
            # so the first matmul's dependency is a single W k-tile.
            WG = 4
            WTv = WT.ap().rearrange("(g j) p n -> g p j n", j=WG)
            wgroups = []
            brow = None
            for g in range(KT // WG):
                w = wpool.tile([P, WG, OSH], bf16, tag=f"w{g}")
                if g == 0:
                    nc.sync.dma_start(w[:, 0:1, :], WTv[0][:, 0:1, :])
                    nc.sync.dma_start(w[:, 1:, :], WTv[0][:, 1:, :])
                else:
                    nc.sync.dma_start(w[:], WTv[g])
                wgroups.append(w)
                if g == 1:
                    brow = cpool.tile([P, OSH], f32)
                    nc.sync.dma_start(brow[:], BIAS[:])
            wtiles = [wgroups[a // WG][:, a % WG, :] for a in range(KT)]

            # bias folded into the PSUM drain (bias row pre-replicated
            # across partitions on host)
            def finish_group(t, gi, acc):
                osb = opool.tile([P, N], f32, tag="osb")
                nc.vector.tensor_tensor(
                    osb[:], acc[:], brow[:, gi * N:(gi + 1) * N],
                    mybir.AluOpType.add)
                nc.scalar.dma_start(
                    OUT.ap()[t * P:(t + 1) * P, gi * N:(gi + 1) * N], osb[:])

            # Phase 1: b-tiles 0-1, k-outer across both column groups, so the
            # PE consumes each W group as it lands (and with 4 matmuls per
            # k-tile vs 0.66us arrival, the stream slack grows every k-tile).
            G = 2
            accs = [[psum.tile([P, N], f32, name=f"acc{t}g{gi}", tag="acc",
                               bufs=5)
                     for gi in range(NG)] for t in range(G)]
            for a in range(KT):
                for t in range(G):
                    for gi in range(NG):
                        nc.tensor.matmul(
                            accs[t][gi][:], xtiles[t][:, a, :],
                            wtiles[a][:, gi * N:(gi + 1) * N],
                            start=(a == 0), stop=(a == KT - 1),
                        )
            for t in range(G):
                for gi in range(NG):
                    finish_group(t, gi, accs[t][gi])

            # Phase 2: remaining b-tiles, k-inner PER COLUMN GROUP, so each
            # group's bias-add + store overlap the next group's 32 matmuls
            # (in particular the second-to-last group's drain hides behind
            # the last group's full pass). The very last group's drain is
            # split in half so even it partially overlaps its own tail.
            for t in range(G, BT):
                load_x(t)
                xsb = xtiles[t]
                for gi in range(NG):
                    if t == BT - 1 and gi == NG - 1:
                        # last group: two half-width accumulators, so half 0's
                        # final matmul (and drain) precede half 1's tail
                        H = N // 2
                        hacc = [psum.tile([P, H], f32, name=f"acch{h}",
                                          tag="acch", bufs=2)
                                for h in range(2)]
                        for a in range(KT):
                            for h in range(2):
                                nc.tensor.matmul(
                                    hacc[h][:], xsb[:, a, :],
                                    wtiles[a][:, gi * N + h * H:
                                               gi * N + (h + 1) * H],
                                    start=(a == 0), stop=(a == KT - 1),
                                )
                        for h in range(2):
                            c0 = gi * N + h * H
                            osb = opool.tile([P, H], f32, tag=f"osbh{h}")
                            nc.vector.tensor_tensor(
                                osb[:], hacc[h][:],
                                brow[:, c0:c0 + H], mybir.AluOpType.add)
                            nc.scalar.dma_start(
                                OUT.ap()[t * P:(t + 1) * P, c0:c0 + H], osb[:])
                        continue
                    acc = psum.tile([P, N], f32, name=f"acc{t}g{gi}",
                                    tag="acc", bufs=5)
                    for a in range(KT):
                        nc.tensor.matmul(
                            acc[:],
                            xsb[:, a, :],       # lhsT: [K=128 (i), M=128 (b)]
                            wtiles[a][:, gi * N:(gi + 1) * N],
                            start=(a == 0),
                            stop=(a == KT - 1),
                        )
                    finish_group(t, gi, acc)

    nc.compile()
    return nc


def kernel(x, weight, bias, indx_seqs):
    x = np.asarray(x, dtype=np.float32)
    weight = np.asarray(weight, dtype=np.float32)
    bias = np.asarray(bias, dtype=np.float32)
    indx_seqs = np.asarray(indx_seqs)

    if "nc" not in _cache:
        _cache["nc"] = _build()
    nc = _cache["nc"]

    # Densify sparse weights: W'[o, i] += weight[o, k] at i = indx_seqs[o, k]
    wd = np.zeros((OUT_F, IN_F), dtype=np.float32)
    np.add.at(wd, (np.arange(OUT_F)[:, None], indx_seqs), weight)

    # Host pre-tiling into SBUF layouts, cast to bf16 (PE runs bf16 at the
    # same 1 cycle/row as fp32r; measured rel_err 3.0e-3 vs the 2e-2 gate).
    # XT[t, p, a, c] = x[t*128+c, a*128+p], 16 global b-tiles.
    xt = np.ascontiguousarray(
        x.reshape(B // P, P, KT, P).transpose(0, 3, 2, 1)
    ).reshape(B // P, P, KT * P).astype(ml_dtypes.bfloat16)
    wts, biases = [], []
    for q in range(MO):
        wshard = wd[q * OSH:(q + 1) * OSH]            # (1024, 4096)
        # WT[a, p, n] = W'[o0+n, a*128+p]
        wts.append(np.ascontiguousarray(
            wshard.reshape(OSH, KT, P).transpose(1, 2, 0)).astype(ml_dtypes.bfloat16))
        biases.append(np.ascontiguousarray(
            np.broadcast_to(bias[q * OSH:(q + 1) * OSH], (P, OSH))))
    in_maps = []
    for c in range(NCORES):
        h, q = divmod(c, MO)
        in_maps.append({
            "XT": xt[h * BT:(h + 1) * BT],
            "WT": wts[q],
            "BIAS": biases[q],
        })

    trace = bool(int(os.environ.get("BASSK_TRACE", "0"))) or bool(
        os.environ.get("BASS_TRACE"))
    if trace:
        _enable_ntff_hook()
    res = run_bass_kernel_spmd(
        nc, in_maps, list(range(NCORES)), trace=trace,
        trace_cores=list(range(NCORES)) if trace else None,
    )
    _cache["last_results"] = res

    # Stitch the 2x4 grid back into the full (B, OUT_F) output.
    out = np.empty((B, OUT_F), dtype=np.float32)
    for c in range(NCORES):
        h, q = divmod(c, MO)
        out[h * BSH:(h + 1) * BSH, q * OSH:(q + 1) * OSH] = res.results[c]["OUT"]
    return out


# revision 25
# speedup vs baseline: 1.0346x; 1.0168x over previous
"""Trainium2 Bass kernel for nn_LinearCondensed.

Computes out[b, o] = sum_k weight[o, k] * x[b, indx_seqs[o, k]] + bias[o]
with B=2048, IN_F=OUT_F=4096, FAN_IN=32.

Strategy: the gather has no fast on-chip primitive (GPSIMD ap_gather measured
~20x below its modeled rate; DMA descriptor gather materializes 32x the data
of x), so we densify the sparse weight matrix on the host --
W'[o, i] = sum_{k: indx_seqs[o,k]==i} weight[o, k] -- and run a dense bf16
matmul out = x @ W'^T + bias on the PE array (1 cycle/row, same as fp32r,
but half the DMA traffic; measured rel_err 3.0e-3 vs the 2e-2 gate; fp8
DoubleRow would be 2x PE but fails the gate at 3-5e-2). OUT_F is sharded
8 ways across cores (512 columns each), x replicated. The kernel is
PE-bound (~110us of streaming at 512 rows/matmul); the single sync HWDGE
queue sustains ~390 GB/s, which keeps every dependency ahead of the PE:
x0, x1, then W in 8 groups (first split 1+3) pace the k-outer phase over
b-tiles 0-1, and x2+ stream during the k-inner phase. Dummy matmuls fill
the ~7us engine-boot head so the PE p-state is fully ramped when real work
arrives; the last b-tile accumulates in two half-width PSUM groups so its
drain overlaps its final matmuls. Bias is folded into the PSUM drain
(pre-replicated across partitions on host). Host pre-tiles both operands
into the exact SBUF layouts so every DMA is a large contiguous copy.
"""

import os
import sys
import types

import ml_dtypes
import numpy as np

import concourse.bacc as bacc
import concourse.mybir as mybir
import concourse.tile as tile
from concourse.bass_utils import run_bass_kernel_spmd

B, IN_F, OUT_F, FAN_IN = 2048, 4096, 4096, 32
NCORES = 8
OSH = OUT_F // NCORES          # 512 output features per core
P = 128                        # partitions
BT = B // P                    # 16 batch tiles
KT = IN_F // P                 # 32 contraction tiles
N = OSH                        # 512 moving columns (max for fp32)

f32 = mybir.dt.float32
f32r = mybir.dt.float32r
bf16 = mybir.dt.bfloat16

_cache = {}


def _enable_ntff_hook():
    """Register the ctypes NTFF profile hook (the image's antenv lacks
    axon_hooks); lets trace=True produce a neuron-profile under axon."""
    try:
        from antenv.axon_hooks import get_axon_ntff_profile_hook  # noqa: F401
        return
    except ImportError:
        pass
    try:
        import antenv
        from trn_agent_boot.trn_boot import _ntff_profile_via_ctypes

        mod = types.ModuleType("antenv.axon_hooks")
        holder = [None]
        mod.set_axon_ntff_profile_hook = lambda h: holder.__setitem__(0, h)
        mod.get_axon_ntff_profile_hook = lambda: holder[0]
        antenv.axon_hooks = mod
        sys.modules["antenv.axon_hooks"] = mod
        mod.set_axon_ntff_profile_hook(
            _ntff_profile_via_ctypes("/opt/axon/libaxon_pjrt.so"))
        import concourse.bass_utils as bu
        bu.upload_artifacts = lambda tmpdir: str(tmpdir)
    except Exception:
        pass


def _build():
    nc = bacc.Bacc()
    # xt[t] is the (128p=i-within-ktile, KT*128=b columns... see layout below)
    # Layouts (host-pretiled, all contiguous):
    #   XT[t, p, a, c] = x[t*128 + c, a*128 + p]   -> per b-tile t: [128, KT*128]
    #   WT[p, a, n]    = W'[o0 + n, a*128 + p]     -> [128, KT*512]
    XT = nc.declare_dram_parameter("XT", [BT, P, KT * P], bf16, isOutput=False)
    WT = nc.declare_dram_parameter("WT", [KT, P, N], bf16, isOutput=False)
    BIAS = nc.declare_dram_parameter("BIAS", [P, N], f32, isOutput=False)
    OUT = nc.declare_dram_parameter("OUT", [B, N], f32, isOutput=True)

    XTv = XT.ap().rearrange("t p (a c) -> t p a c", a=KT)

    with tile.TileContext(nc) as tc:
        with (
            tc.tile_pool(name="wpool", bufs=1) as wpool,
            tc.tile_pool(name="xpool", bufs=4) as xpool,
            tc.tile_pool(name="cpool", bufs=1) as cpool,
            tc.tile_pool(name="opool", bufs=3) as opool,
            tc.tile_pool(name="psum", bufs=4, space="PSUM") as psum,
        ):
            # All input loads ride the single sync HWDGE FIFO in a deliberate
            # order: x0, x1 at full bandwidth (PE can start at ~6us), then
            # the 32 weight k-tiles (which pace b-tile 0), then x2+ arrive
            # just in time. Output stores use the scalar HWDGE queue so they
            # never block input loads.
            xtiles = {}

            # The PE p-state ramps to 2.4GHz only after ~3us of continuous
            # work; real matmuls can't start until x0+x1+wg0 land (~15us).
            # Fill the idle head with dummy matmuls on zeroed SBUF so the
            # array is warm (and the pipeline primed) when real work arrives.
            dl = cpool.tile([P, P], bf16)
            dr = cpool.tile([P, N], bf16)
            nc.vector.memset(dl[:], 0)
            nc.vector.memset(dr[:], 0)
            dacc = psum.tile([P, N], f32, name="dacc", tag="dacc", bufs=1)
            for _ in range(20):
                nc.tensor.matmul(dacc[:], dl[:], dr[:], start=True, stop=True)

            def load_x(t):
                xs = xpool.tile([P, KT, P], bf16, tag="xs")
                nc.sync.dma_start(xs[:], XTv[t])
                xtiles[t] = xs

            load_x(0)
            load_x(1)
            # weights in 8 groups of 4 k-tiles (1MB per DMA): few enough
            # triggers (~0.65us each on the issuing engine) to not serialize
            # the start, fine-grained enough to pace b-tile 0.
            WG = 4
            wgroups = []
            brow = ones = None
            WTv = WT.ap().rearrange("(g j) p n -> g p j n", j=WG)
            for g in range(KT // WG):
                w = wpool.tile([P, WG, N], bf16, tag=f"w{g}")
                if g == 0:
                    # first group split 1+3 so the first matmul's dependency
                    # is x0+x1+one W k-tile (2.125MB) instead of four (2.5MB)
                    nc.sync.dma_start(w[:, 0:1, :], WTv[0][:, 0:1, :])
                    nc.sync.dma_start(w[:, 1:, :], WTv[0][:, 1:, :])
                else:
                    nc.sync.dma_start(w[:], WTv[g])
                wgroups.append(w)
                if g == 1:
                    brow = cpool.tile([P, N], f32)
                    nc.sync.dma_start(brow[:], BIAS[:])
            wtiles = [wgroups[a // WG][:, a % WG, :] for a in range(KT)]

            # bias folded into the PSUM drain: osb = acc + bias (bias row
            # pre-replicated across partitions on host), saving 16 K=1 bias
            # matmuls on the PE.
            def finish_tile(t, acc):
                osb = opool.tile([P, N], f32, tag="osb")
                nc.vector.tensor_tensor(osb[:], acc[:], brow[:], mybir.AluOpType.add)
                nc.scalar.dma_start(OUT.ap()[t * P:(t + 1) * P, :], osb[:])

            # Phase 1: b-tiles 0-1 in k-outer order so the PE consumes each
            # weight group as it lands instead of idling through the 8MB
            # weight stream.
            G = 2
            accs = [psum.tile([P, N], f32, name=f"acc{t}", tag="acc")
                    for t in range(G)]
            for a in range(KT):
                for t in range(G):
                    nc.tensor.matmul(
                        accs[t][:], xtiles[t][:, a, :], wtiles[a][:],
                        start=(a == 0), stop=(a == KT - 1),
                    )
            for t in range(G):
                finish_tile(t, accs[t])

            # Phase 2: remaining b-tiles, k-inner, x streamed just in time.
            for t in range(G, BT - 1):
                load_x(t)
                xsb = xtiles[t]
                acc = psum.tile([P, N], f32, tag="acc")
                for a in range(KT):
                    nc.tensor.matmul(
                        acc[:],
                        xsb[:, a, :],      # lhsT: [K=128 (i), M=128 (b)]
                        wtiles[a][:],      # rhs:  [K=128 (i), N=512 (o)]
                        start=(a == 0),
                        stop=(a == KT - 1),
                    )
                finish_tile(t, acc)

            # Last b-tile: split into two 256-column accumulation groups so
            # the first half's bias-add + store overlap the second half's
            # final matmuls, shortening the drain tail after the last matmul.
            t = BT - 1
            load_x(t)
            xsb = xtiles[t]
            H = N // 2
            acc_h = [psum.tile([P, H], f32, name=f"acch{h}", tag="acch", bufs=2)
                     for h in range(2)]
            for a in range(KT):
                for h in range(2):
                    nc.tensor.matmul(
                        acc_h[h][:], xsb[:, a, :],
                        wtiles[a][:, h * H:(h + 1) * H],
                        start=(a == 0), stop=(a == KT - 1),
                    )
            for h in range(2):
                osb = opool.tile([P, H], f32, tag=f"osbh{h}")
                nc.vector.tensor_tensor(
                    osb[:], acc_h[h][:], brow[:, h * H:(h + 1) * H],
                    mybir.AluOpType.add)
                nc.scalar.dma_start(
                    OUT.ap()[t * P:(t + 1) * P, h * H:(h + 1) * H], osb[:])

    nc.compile()
    return nc


def kernel(x, weight, bias, indx_seqs):
    x = np.asarray(x, dtype=np.float32)
    weight = np.asarray(weight, dtype=np.float32)
    bias = np.asarray(bias, dtype=np.float32)
    indx_seqs = np.asarray(indx_seqs)

    if "nc" not in _cache:
        _cache["nc"] = _build()
    nc = _cache["nc"]

    # Densify sparse weights: W'[o, i] += weight[o, k] at i = indx_seqs[o, k]
    wd = np.zeros((OUT_F, IN_F), dtype=np.float32)
    np.add.at(wd, (np.arange(OUT_F)[:, None], indx_seqs), weight)

    # Host pre-tiling into SBUF-friendly layouts, cast to bf16 (the PE runs
    # bf16 at the same 1 cycle/row as fp32r, so this halves DMA traffic at a
    # measured cost of rel_err 3.0e-3 vs the 2e-2 gate).
    # XT[t, p, a, c] = x[t*128+c, a*128+p]
    xt = np.ascontiguousarray(
        x.reshape(BT, P, KT, P).transpose(0, 3, 2, 1)
    ).reshape(BT, P, KT * P).astype(ml_dtypes.bfloat16)
    in_maps = []
    for c in range(NCORES):
        wshard = wd[c * OSH:(c + 1) * OSH]            # (512, 4096)
        # WT[a, p, n] = W'[o0+n, a*128+p]
        wt = np.ascontiguousarray(
            wshard.reshape(OSH, KT, P).transpose(1, 2, 0)).astype(ml_dtypes.bfloat16)
        in_maps.append({
            "XT": xt,
            "WT": wt,
            "BIAS": np.ascontiguousarray(np.broadcast_to(bias[c * OSH:(c + 1) * OSH], (P, N))),
        })

    trace = bool(int(os.environ.get("BASSK_TRACE", "0"))) or bool(
        os.environ.get("BASS_TRACE"))
    if trace:
        _enable_ntff_hook()
    res = run_bass_kernel_spmd(
        nc, in_maps, list(range(NCORES)), trace=trace,
        trace_cores=list(range(NCORES)) if trace else None,
    )
    _cache["last_results"] = res

    out = np.concatenate([res.results[c]["OUT"] for c in range(NCORES)], axis=1)
    return out



# revision 26
# speedup vs baseline: 1.0394x; 1.0046x over previous
"""Trainium2 Bass kernel for nn_LinearCondensed.

Computes out[b, o] = sum_k weight[o, k] * x[b, indx_seqs[o, k]] + bias[o]
with B=2048, IN_F=OUT_F=4096, FAN_IN=32.

Strategy: the gather has no fast on-chip primitive (GPSIMD ap_gather measured
~20x below its modeled rate; DMA descriptor gather materializes 32x the data
of x), so we densify the sparse weight matrix on the host --
W'[o, i] = sum_{k: indx_seqs[o,k]==i} weight[o, k] -- and run a dense bf16
matmul out = x @ W'^T + bias on the PE array (1 cycle/row, same as fp32r,
but half the DMA traffic; measured rel_err 3.0e-3 vs the 2e-2 gate; fp8
DoubleRow would be 2x PE but fails the gate at 3-5e-2). OUT_F is sharded
8 ways across cores (512 columns each), x replicated. The kernel is
PE-bound (~110us of streaming at 512 rows/matmul); the single sync HWDGE
queue sustains ~390 GB/s, which keeps every dependency ahead of the PE:
x0, x1, then W in 8 groups (first split 1+3) pace the k-outer phase over
b-tiles 0-1, and x2+ stream during the k-inner phase. Dummy matmuls fill
the ~7us engine-boot head so the PE p-state is fully ramped when real work
arrives; the last b-tile accumulates in two half-width PSUM groups so its
drain overlaps its final matmuls. Bias is folded into the PSUM drain
(pre-replicated across partitions on host). Host pre-tiles both operands
into the exact SBUF layouts so every DMA is a large contiguous copy.
"""

import os
import sys
import types

import ml_dtypes
import numpy as np

import concourse.bacc as bacc
import concourse.mybir as mybir
import concourse.tile as tile
from concourse.bass_utils import run_bass_kernel_spmd

B, IN_F, OUT_F, FAN_IN = 2048, 4096, 4096, 32
NCORES = 8
OSH = OUT_F // NCORES          # 512 output features per core
P = 128                        # partitions
BT = B // P                    # 16 batch tiles
KT = IN_F // P                 # 32 contraction tiles
N = OSH                        # 512 moving columns (max for fp32)

f32 = mybir.dt.float32
f32r = mybir.dt.float32r
bf16 = mybir.dt.bfloat16

_cache = {}


def _enable_ntff_hook():
    """Register the ctypes NTFF profile hook (the image's antenv lacks
    axon_hooks); lets trace=True produce a neuron-profile under axon."""
    try:
        from antenv.axon_hooks import get_axon_ntff_profile_hook  # noqa: F401
        return
    except ImportError:
        pass
    try:
        import antenv
        from trn_agent_boot.trn_boot import _ntff_profile_via_ctypes

        mod = types.ModuleType("antenv.axon_hooks")
        holder = [None]
        mod.set_axon_ntff_profile_hook = lambda h: holder.__setitem__(0, h)
        mod.get_axon_ntff_profile_hook = lambda: holder[0]
        antenv.axon_hooks = mod
        sys.modules["antenv.axon_hooks"] = mod
        mod.set_axon_ntff_profile_hook(
            _ntff_profile_via_ctypes("/opt/axon/libaxon_pjrt.so"))
        import concourse.bass_utils as bu
        bu.upload_artifacts = lambda tmpdir: str(tmpdir)
    except Exception:
        pass


def _build():
    nc = bacc.Bacc()
    # xt[t] is the (128p=i-within-ktile, KT*128=b columns... see layout below)
    # Layouts (host-pretiled, all contiguous):
    #   XT[t, p, a, c] = x[t*128 + c, a*128 + p]   -> per b-tile t: [128, KT*128]
    #   WT[p, a, n]    = W'[o0 + n, a*128 + p]     -> [128, KT*512]
    XT = nc.declare_dram_parameter("XT", [BT, P, KT * P], bf16, isOutput=False)
    WT = nc.declare_dram_parameter("WT", [KT, P, N], bf16, isOutput=False)
    BIAS = nc.declare_dram_parameter("BIAS", [P, N], f32, isOutput=False)
    OUT = nc.declare_dram_parameter("OUT", [B, N], f32, isOutput=True)

    XTv = XT.ap().rearrange("t p (a c) -> t p a c", a=KT)

    with tile.TileContext(nc) as tc:
        with (
            tc.tile_pool(name="wpool", bufs=1) as wpool,
            tc.tile_pool(name="xpool", bufs=4) as xpool,
            tc.tile_pool(name="cpool", bufs=1) as cpool,
            tc.tile_pool(name="opool", bufs=3) as opool,
            tc.tile_pool(name="psum", bufs=4, space="PSUM") as psum,
        ):
            # All input loads ride the single sync HWDGE FIFO in a deliberate
            # order: x0, x1 at full bandwidth (PE can start at ~6us), then
            # the 32 weight k-tiles (which pace b-tile 0), then x2+ arrive
            # just in time. Output stores use the scalar HWDGE queue so they
            # never block input loads.
            xtiles = {}

            # The PE p-state ramps to 2.4GHz only after ~3us of continuous
            # work; real matmuls can't start until x0+x1+wg0 land (~15us).
            # Fill the idle head with dummy matmuls on zeroed SBUF so the
            # array is warm (and the pipeline primed) when real work arrives.
            dl = cpool.tile([P, P], bf16)
            dr = cpool.tile([P, N], bf16)
            nc.vector.memset(dl[:], 0)
            nc.vector.memset(dr[:], 0)
            dacc = psum.tile([P, N], f32, name="dacc", tag="dacc", bufs=1)
            for _ in range(20):
                nc.tensor.matmul(dacc[:], dl[:], dr[:], start=True, stop=True)

            def load_x(t):
                xs = xpool.tile([P, KT, P], bf16, tag="xs")
                nc.sync.dma_start(xs[:], XTv[t])
                xtiles[t] = xs

            load_x(0)
            load_x(1)
            # weights in 8 groups of 4 k-tiles (1MB per DMA): few enough
            # triggers (~0.65us each on the issuing engine) to not serialize
            # the start, fine-grained enough to pace b-tile 0.
            WG = 4
            wgroups = []
            brow = ones = None
            WTv = WT.ap().rearrange("(g j) p n -> g p j n", j=WG)
            for g in range(KT // WG):
                w = wpool.tile([P, WG, N], bf16, tag=f"w{g}")
                if g == 0:
                    # k-tiles 0-3 as 1+1+2: each chunk's semaphore fires just
                    # ahead of the PE's 0.43us/k-tile consumption (a single
                    # 4-k-tile DMA sems 0.7us after k-tile 1 is needed)
                    nc.sync.dma_start(w[:, 0:1, :], WTv[0][:, 0:1, :])
                    nc.sync.dma_start(w[:, 1:2, :], WTv[0][:, 1:2, :])
                    nc.sync.dma_start(w[:, 2:, :], WTv[0][:, 2:, :])
                elif g in (1, 2):
                    # k-tiles 4-11 in 2-k-tile halves: arrival still leads
                    # consumption while the startup deficit drains
                    nc.sync.dma_start(w[:, 0:2, :], WTv[g][:, 0:2, :])
                    nc.sync.dma_start(w[:, 2:, :], WTv[g][:, 2:, :])
                else:
                    nc.sync.dma_start(w[:], WTv[g])
                wgroups.append(w)
                if g == 1:
                    brow = cpool.tile([P, N], f32)
                    nc.sync.dma_start(brow[:], BIAS[:])
            wtiles = [wgroups[a // WG][:, a % WG, :] for a in range(KT)]

            # bias folded into the PSUM drain: osb = acc + bias (bias row
            # pre-replicated across partitions on host), saving 16 K=1 bias
            # matmuls on the PE.
            def finish_tile(t, acc):
                osb = opool.tile([P, N], f32, tag="osb")
                nc.vector.tensor_tensor(osb[:], acc[:], brow[:], mybir.AluOpType.add)
                nc.scalar.dma_start(OUT.ap()[t * P:(t + 1) * P, :], osb[:])

            # Phase 1: b-tiles 0-1 in k-outer order so the PE consumes each
            # weight group as it lands instead of idling through the 8MB
            # weight stream.
            G = 2
            accs = [psum.tile([P, N], f32, name=f"acc{t}", tag="acc")
                    for t in range(G)]
            for a in range(KT):
                for t in range(G):
                    nc.tensor.matmul(
                        accs[t][:], xtiles[t][:, a, :], wtiles[a][:],
                        start=(a == 0), stop=(a == KT - 1),
                    )
            for t in range(G):
                finish_tile(t, accs[t])

            # Phase 2: remaining b-tiles, k-inner, x streamed just in time.
            for t in range(G, BT - 1):
                load_x(t)
                xsb = xtiles[t]
                acc = psum.tile([P, N], f32, tag="acc")
                for a in range(KT):
                    nc.tensor.matmul(
                        acc[:],
                        xsb[:, a, :],      # lhsT: [K=128 (i), M=128 (b)]
                        wtiles[a][:],      # rhs:  [K=128 (i), N=512 (o)]
                        start=(a == 0),
                        stop=(a == KT - 1),
                    )
                finish_tile(t, acc)

            # Last b-tile: split into two 256-column accumulation groups so
            # the first half's bias-add + store overlap the second half's
            # final matmuls, shortening the drain tail after the last matmul.
            t = BT - 1
            load_x(t)
            xsb = xtiles[t]
            H = N // 2
            acc_h = [psum.tile([P, H], f32, name=f"acch{h}", tag="acch", bufs=2)
                     for h in range(2)]
            for a in range(KT):
                for h in range(2):
                    nc.tensor.matmul(
                        acc_h[h][:], xsb[:, a, :],
                        wtiles[a][:, h * H:(h + 1) * H],
                        start=(a == 0), stop=(a == KT - 1),
                    )
            for h in range(2):
                osb = opool.tile([P, H], f32, tag=f"osbh{h}")
                nc.vector.tensor_tensor(
                    osb[:], acc_h[h][:], brow[:, h * H:(h + 1) * H],
                    mybir.AluOpType.add)
                nc.scalar.dma_start(
                    OUT.ap()[t * P:(t + 1) * P, h * H:(h + 1) * H], osb[:])

    nc.compile()
    return nc


def kernel(x, weight, bias, indx_seqs):
    x = np.asarray(x, dtype=np.float32)
    weight = np.asarray(weight, dtype=np.float32)
    bias = np.asarray(bias, dtype=np.float32)
    indx_seqs = np.asarray(indx_seqs)

    if "nc" not in _cache:
        _cache["nc"] = _build()
    nc = _cache["nc"]

    # Densify sparse weights: W'[o, i] += weight[o, k] at i = indx_seqs[o, k]
    wd = np.zeros((OUT_F, IN_F), dtype=np.float32)
    np.add.at(wd, (np.arange(OUT_F)[:, None], indx_seqs), weight)

    # Host pre-tiling into SBUF-friendly layouts, cast to bf16 (the PE runs
    # bf16 at the same 1 cycle/row as fp32r, so this halves DMA traffic at a
    # measured cost of rel_err 3.0e-3 vs the 2e-2 gate).
    # XT[t, p, a, c] = x[t*128+c, a*128+p]
    xt = np.ascontiguousarray(
        x.reshape(BT, P, KT, P).transpose(0, 3, 2, 1)
    ).reshape(BT, P, KT * P).astype(ml_dtypes.bfloat16)
    in_maps = []
    for c in range(NCORES):
        wshard = wd[c * OSH:(c + 1) * OSH]            # (512, 4096)
        # WT[a, p, n] = W'[o0+n, a*128+p]
        wt = np.ascontiguousarray(
            wshard.reshape(OSH, KT, P).transpose(1, 2, 0)).astype(ml_dtypes.bfloat16)
        in_maps.append({
            "XT": xt,
            "WT": wt,
            "BIAS": np.ascontiguousarray(np.broadcast_to(bias[c * OSH:(c + 1) * OSH], (P, N))),
        })

    trace = bool(int(os.environ.get("BASSK_TRACE", "0"))) or bool(
        os.environ.get("BASS_TRACE"))
    if trace:
        _enable_ntff_hook()
    res = run_bass_kernel_spmd(
        nc, in_maps, list(range(NCORES)), trace=trace,
        trace_cores=list(range(NCORES)) if trace else None,
    )
    _cache["last_results"] = res

    out = np.concatenate([res.results[c]["OUT"] for c in range(NCORES)], axis=1)
    return out

